# revision 30
# baseline (speedup 1.0000x reference)
"""Trainium2 Bass kernel for nn_Encoder_78649441124984.

Encoder: pos_emb + 4x(sepconv+res) + MHA(+res) + ffc(+res).
Sharding: data-parallel over batch, 8 cores x 4 batch elements, all
parameters replicated; no collectives.

Layout: activations as [feature, time] tiles ([128, 512] SBUF, feature on
partitions), residual stream in float32r which feeds the PE directly
(f32r moving operands >=256 cols run at 1 cycle/col, same as bf16, so no
low-precision casts exist anywhere in the main path).

Depthwise conv runs as per-partition scalar_tensor_tensor tap chains
spread across DVE / Pool / PE-diag-matmul units (static assignment per
(layer, block) tuned against the engine budget); pointwise/qkv/out/ffc
are f32r matmuls; attention uses the transposed-scores scheme with a
ones column smuggled into v^T for softmax sums, normalized via
gather/recip/broadcast matmuls.
"""
import sys

sys.path.insert(0, "/opt/trn_rl_repo")

import numpy as np
import ml_dtypes

import concourse.bass as bass
import concourse.mybir as mybir
import concourse.tile as tile
from concourse import bacc
from concourse.bass_utils import run_bass_kernel_spmd

F32 = mybir.dt.float32
F32R = mybir.dt.float32r
BF16 = mybir.dt.bfloat16
AF = mybir.ActivationFunctionType
ALU = mybir.AluOpType

D = 500
H = 10
HD = 50
B, T = 32, 512
K = 7
NC_ = 8
BS = B // NC_          # batch shard per core
DP = 512               # padded feature dim
CT = 4                 # feature tiles (4 x 128 = 512 >= 500)
HP = 640               # padded head dim total (10 heads x 64 slots)

# depthwise engine plan: PE_UNITS are (layer, blk) pairs whose whole 7-tap
# chain runs as f32r diagonal matmuls on the PE (reading the halo'd xw);
# remaining blocks run DVE stt chains with POOL_TAPS offloaded to Pool
# (range-nested tt chains). Taps index k in 0..6, shift s = k-3.
PE_UNITS = {(0, 0), (1, 0), (2, 0)}
POOL_TAPS = {0: [1, 0], 1: [1, 0], 2: [1], 3: [1]}
DVE_TAPS = {l: [3] + [k for k in (2, 0, 1, 4, 5, 6) if k not in POOL_TAPS[l]]
            for l in range(4)}
PW_FP8 = True
TB = T + 8             # halo'd block stride in the residual wide tiles
FP8 = mybir.dt.float8e4


def _rows(ct):
    return min(128, D - 128 * ct)


def _head_col(h):
    return 128 * (h // 2) + 64 * (h % 2)


def build_host_consts(dw, pw, db, pb, in_w, in_b, out_w, out_b, ffc_w, ffc_b):
    """Pack all weights into device layouts. dw: [4][D,1,K], pw: [4][D,D]."""
    c = {}
    for l in range(4):
        t = np.zeros((DP, DP), np.float32)
        t[:D, :D] = pw[l].T
        c[f"pwT{l}"] = t
    c["ident"] = np.eye(128, dtype=np.float32)
    for (l, blk) in sorted(PE_UNITS):
        r0, r1 = 128 * blk, min(D, 128 * blk + 128)
        dg = np.zeros((K, 128, 128), np.float32)
        for k in range(K):
            dcol = np.zeros((128,), np.float32)
            dcol[0:r1 - r0] = dw[l][r0:r1, 0, k]
            np.fill_diagonal(dg[k], dcol)
        c[f"diag{l}_{blk}"] = dg.transpose(1, 0, 2).reshape(128, -1)
    # qkv in-proj, transposed, head-pair padded. q gets the 1/sqrt(HD) scale.
    scale = HD ** -0.5
    inwT = np.zeros((DP, 2 * HP), np.float32)
    inb_cols = np.zeros((128, 10), np.float32)
    for h in range(H):
        base = _head_col(h)
        qrows = slice(100 * (h // 2) + 50 * (h % 2),
                      100 * (h // 2) + 50 * (h % 2) + 50)
        inwT[:D, base:base + 50] = in_w[qrows, :].T * scale
        inb_cols[base % 128:base % 128 + 50, h // 2] = in_b[qrows] * scale
        krows = slice(500 + qrows.start, 500 + qrows.stop)
        inwT[:D, HP + base:HP + base + 50] = in_w[krows, :].T
        inb_cols[base % 128:base % 128 + 50, 5 + h // 2] = in_b[krows]
    c["inwT"] = inwT
    c["inb_cols"] = inb_cols
    # v in-proj (produces v^T directly); ones column handled on device
    wv = np.zeros((DP, HP), np.float32)
    crow = np.zeros((1, HP), np.float32)
    for h in range(H):
        base = _head_col(h)
        vrows = slice(1000 + 50 * h, 1000 + 50 * h + 50)
        wv[:D, base:base + 50] = in_w[vrows, :].T
        crow[0, base:base + 50] = in_b[vrows]
        crow[0, base + 50] = 1.0
    c["wv"] = wv
    c["crow"] = crow.astype(np.float32)
    # out-proj: owT[hd_pad, e]
    owT = np.zeros((HP, DP), np.float32)
    for h in range(H):
        base = _head_col(h)
        owT[base:base + 50, :D] = out_w[:, 50 * h:50 * h + 50].T
    c["owT"] = owT
    c["outb_col"] = np.pad(out_b, (0, DP - D)).reshape(CT, 128).T.astype(np.float32)
    c["ffcT"] = np.pad(ffc_w.T, ((0, DP - D), (0, DP - D))).astype(np.float32)
    c["ffcb_col"] = np.pad(ffc_b, (0, DP - D)).reshape(CT, 128).T.astype(np.float32)
    # gather / broadcast selectors for softmax normalization
    G = np.zeros((5 * 128, H), np.float32)
    E = np.zeros((5 * H, 128), np.float32)
    for p in range(5):
        G[128 * p + 50, 2 * p] = 1.0
        G[128 * p + 114, 2 * p + 1] = 1.0
        E[H * p + 2 * p, 0:50] = 1.0
        E[H * p + 2 * p + 1, 64:114] = 1.0
    c["G"] = G
    c["E"] = E
    # constant positional-embedding table, transposed: peT[d, t]
    half = D // 2
    inv = np.exp(np.arange(half, dtype=np.float64) * (-np.log(10000.0) / (half - 1)))
    pos = np.arange(1, T + 1, dtype=np.float64)
    ang = pos[None, :] * inv[:, None]
    peT = np.zeros((DP, TB), np.float32)
    peT[:half, 4:4 + T] = np.sin(ang)
    peT[half:D, 4:4 + T] = np.cos(ang)
    c["peT"] = peT.astype(np.float32)
    # depthwise tap weight columns: [128, 4L*4blk*K]
    dwc = np.zeros((128, 4 * CT * K), np.float32)
    for l in range(4):
        for blk in range(CT):
            r0, r1 = 128 * blk, min(D, 128 * blk + 128)
            for k in range(K):
                dwc[0:r1 - r0, (l * CT + blk) * K + k] = dw[l][r0:r1, 0, k]
    c["dwc"] = dwc
    # per-layer conv biases (zeros in this model; applied only when nonzero).
    # depthwise bias folds through the pointwise conv: pb_eff = pb + pw @ db
    c["db_cols"] = np.stack(
        [np.pad(db[l], (0, DP - D)).reshape(CT, 128).T for l in range(4)], 0)
    c["pb_cols"] = np.stack(
        [np.pad(pb[l] + pw[l] @ db[l], (0, DP - D)).reshape(CT, 128).T
         for l in range(4)], 0)
    return _pack_consts(c)


def _rt(a):
    """Repack row-tiled [n*128, C] -> [128, n*C] (tile ct at cols ct*C)."""
    n = a.shape[0] // 128
    return a.reshape(n, 128, a.shape[1]).transpose(1, 0, 2).reshape(128, -1)


def _pack_consts(c):
    """Coalesce all [128, *] f32r weight tiles into one contiguous wall."""
    sections = []
    offs = {}
    w = 0

    def add(name, arr):
        nonlocal w
        offs[name] = w
        sections.append(arr.astype(np.float32))
        w += arr.shape[1]

    add("ident", c.pop("ident"))
    for (l, blk) in sorted(PE_UNITS):
        add(f"diag{l}_{blk}", c.pop(f"diag{l}_{blk}"))
    add("inwT", _rt(c.pop("inwT")))
    add("wv", _rt(c.pop("wv")))
    add("ffcT", _rt(c.pop("ffcT")))
    wall = np.concatenate(sections, 1)
    sections16 = []
    offs16 = {}
    w16 = 0
    pw8 = []
    for l in range(4):
        a = _rt(c.pop(f"pwT{l}"))          # [128, 4ct*DP] (cin-tile, cout)
        if PW_FP8:
            # DoubleRow pairs: [pair pr][128, 2, 512] planes = cin tiles
            b_ = a.reshape(128, CT, DP)
            for pr in range(2):
                pw8.append(np.stack([b_[:, 2 * pr], b_[:, 2 * pr + 1]], 1)
                           .reshape(128, 2 * DP))
        else:
            offs16[f"pwT{l}"] = w16
            sections16.append(a)
            w16 += a.shape[1]
    for nm in ("owT", "G"):
        a = _rt(c.pop(nm))
        offs16[nm] = w16
        sections16.append(a)
        w16 += a.shape[1]
    wall16 = np.concatenate(sections16, 1).astype(ml_dtypes.bfloat16)
    wall8 = (np.concatenate(pw8, 1).astype(ml_dtypes.float8_e4m3fn)
             if PW_FP8 else np.zeros((128, 1), ml_dtypes.float8_e4m3fn))
    sm = np.concatenate(
        [c.pop("inb_cols"), c.pop("outb_col"), c.pop("ffcb_col"),
         np.concatenate(list(c.pop("db_cols")), 1),
         np.concatenate(list(c.pop("pb_cols")), 1),
         c.pop("dwc")], 1).astype(np.float32)
    E = c.pop("E")
    out = {
        "wall": wall,
        "wall16": wall16,
        "wall8": wall8,
        "smallf": sm,
        "peTp": _rt(c.pop("peT")).astype(ml_dtypes.bfloat16),
        "E_all": np.ascontiguousarray(
            np.concatenate([E[10 * p:10 * (p + 1), :] for p in range(5)], 1)).astype(np.float32),
        "crow": c.pop("crow"),
        "_offs": offs,
        "_offs16": offs16,
    }
    return out


def trace_program(consts, mask_any, bias_any, pad_any):
    """Build the SPMD Bass program (same for every core)."""
    nc = bacc.Bacc("TRN2", target_bir_lowering=False, debug=False,
                   num_devices=NC_)

    xT_d = nc.dram_tensor("xT", [BS, D, T], F32, kind="ExternalInput")
    orix_d = nc.dram_tensor("orix", [BS, T], mybir.dt.int32, kind="ExternalInput")
    xmask_d = nc.dram_tensor("xmask", [BS, T], mybir.dt.uint8, kind="ExternalInput")
    out_d = nc.dram_tensor("out", [BS, D, T], F32, kind="ExternalOutput")

    wd = {"_offs": consts["_offs"], "_offs16": consts["_offs16"]}
    for name, arr in consts.items():
        if name.startswith("_offs"):
            continue
        if name in ("wall", "E_all"):
            dt = F32R
        elif name in ("wall16", "peTp"):
            dt = BF16
        elif name == "wall8":
            dt = FP8
        else:
            dt = F32
        wd[name] = nc.dram_tensor(name, list(arr.shape), dt, kind="ExternalInput")

    with tile.TileContext(nc, num_cores=NC_) as tc:
        _trace_body(nc, tc, wd, xT_d, orix_d, xmask_d, out_d, mask_any,
                    bias_any, pad_any)
    nc.finalize()
    return nc


def _trace_body(nc, tc, wd, xT_d, orix_d, xmask_d, out_d, mask_any, bias_any, pad_any):
    from contextlib import ExitStack
    ctx = ExitStack()
    with ctx:
        wpool = ctx.enter_context(tc.tile_pool(name="w", bufs=1))
        offs = wd["_offs"]
        Wtot = wd["wall"].shape[1]
        wall_t = wpool.tile([128, Wtot], F32R, tag="wall", name="wall")
        peTp = wpool.tile([128, CT * TB], BF16, tag="peTp", name="peTp")
        smallf = wpool.tile([128, 50 + 4 * CT * K], F32, tag="smallf", name="smallf")
        E_t = wpool.tile([H, 5 * 128], F32R, tag="E_t", name="E_t")
        offs16 = wd["_offs16"]
        W16 = wd["wall16"].shape[1]
        wall16_t = wpool.tile([128, W16], BF16, tag="wall16", name="wall16")
        W8 = wd["wall8"].shape[1]
        wall8_t = wpool.tile([128, W8], FP8, tag="wall8", name="wall8")
        crow_t = wpool.tile([1, HP], F32, tag="crow", name="crow")
        sec16_w = {}
        for s16 in offs16:
            nxt = [offs16[t] for t in offs16 if offs16[t] > offs16[s16]]
            sec16_w[s16] = (min(nxt) if nxt else W16) - offs16[s16]
        sec_w = {}
        for s in offs:
            nxt = [offs[t] for t in offs if offs[t] > offs[s]]
            sec_w[s] = (min(nxt) if nxt else Wtot) - offs[s]

        def dma16(eng, s16):
            eng.dma_start(
                wall16_t[:, offs16[s16]:offs16[s16] + sec16_w[s16]],
                wd["wall16"][:, offs16[s16]:offs16[s16] + sec16_w[s16]])

        def dmaw(eng, s):
            eng.dma_start(wall_t[:, offs[s]:offs[s] + sec_w[s]],
                          wd["wall"][:, offs[s]:offs[s] + sec_w[s]])

        # critical sections before the first element's prologue
        dmaw(nc.sync, "ident")

        # pos-emb table + tap weights are read by the very first chunk:
        # their DMAs must precede it in trace order
        nc.scalar.dma_start(peTp[:], wd["peTp"][:])
        nc.scalar.dma_start(smallf[:], wd["smallf"][:])

        def emit_consts():
            nc.scalar.dma_start(E_t[:], wd["E_all"][:])
            nc.scalar.dma_start(crow_t[:], wd["crow"][:])

        def emit_wall_bulk():
            if PW_FP8:
                nc.sync.dma_start(wall8_t[:], wd["wall8"][:])
            else:
                dma16(nc.sync, "pwT0")
                dma16(nc.gpsimd, "pwT1")
                dma16(nc.sync, "pwT2")
                dma16(nc.gpsimd, "pwT3")
            for (l_, blk_) in sorted(PE_UNITS):
                dmaw(nc.gpsimd, f"diag{l_}_{blk_}")
            dmaw(nc.gpsimd, "wv")
            dmaw(nc.sync, "inwT")
            dmaw(nc.gpsimd, "ffcT")
            dma16(nc.sync, "owT")
            dma16(nc.sync, "G")
        if bias_any:
            C_t = wpool.tile([128, HP], F32, tag="C", name="C")
            nc.gpsimd.partition_broadcast(C_t[:], crow_t[:])
        else:
            C_t = None

        def wsl(name, a, w):
            o = offs[name] + a
            return wall_t[:, o:o + w]

        def wsl16(name, a, w):
            o = offs16[name] + a
            return wall16_t[:, o:o + w]

        ident = wsl("ident", 0, 128)
        diag = {(l, blk): [wsl(f"diag{l}_{blk}", 128 * k, 128) for k in range(K)]
                for (l, blk) in PE_UNITS}
        if PW_FP8:
            # pw8 pair lhsT [128, 2, 128-cout-block]: pair pr of layer l at
            # columns (l*2+pr)*2*DP, planes at stride DP, cout block at 128
            def pw8sl(l, pr, ot):
                o = (l * 2 + pr) * 2 * DP + 128 * ot
                a = wall8_t[:, o:o + DP + 128]
                return bass.AP(a.tensor, a.offset, [a.ap[0], [DP, 2], [1, 128]])
            pwT = None
        else:
            pwT = [[wsl16(f"pwT{l}", DP * ct, DP) for ct in range(CT)]
                   for l in range(4)]
        inwT = [wsl("inwT", 2 * HP * ct, 2 * HP) for ct in range(CT)]
        wv = [wsl("wv", HP * ct, HP) for ct in range(CT)]
        owT = [wsl16("owT", DP * p, DP) for p in range(5)]
        ffcT = [wsl("ffcT", DP * ct, DP) for ct in range(CT)]
        G = [wsl16("G", H * p, H) for p in range(5)]
        E = [E_t[:, 128 * p:128 * (p + 1)] for p in range(5)]
        peT = [peTp[:, T * ct:T * (ct + 1)] for ct in range(CT)]
        inb_cols = smallf[:, 0:10]
        outb_col = smallf[:, 10:14]
        ffcb_col = smallf[:, 14:18]
        db_cols = [smallf[:, 18 + CT * l:18 + CT * (l + 1)] for l in range(4)]
        pb_cols = [smallf[:, 34 + CT * l:34 + CT * (l + 1)] for l in range(4)]
        dwc = smallf[:, 50:50 + 4 * CT * K]

        # ---- per-batch-element pools ----
        xpool = ctx.enter_context(tc.tile_pool(name="x", bufs=3))
        apool = ctx.enter_context(tc.tile_pool(name="acc", bufs=2))
        mpool = ctx.enter_context(tc.tile_pool(name="m", bufs=2))
        qkpool = ctx.enter_context(tc.tile_pool(name="qk", bufs=1))
        epool = ctx.enter_context(tc.tile_pool(name="e", bufs=2))
        opool = ctx.enter_context(tc.tile_pool(name="o", bufs=1))
        pp = ctx.enter_context(tc.tile_pool(name="pp", bufs=6, space="PSUM"))
        pa = ctx.enter_context(tc.tile_pool(name="pa", bufs=1, space="PSUM"))

        pwsl = pw8sl if PW_FP8 else None
        gens = [
            _trace_batch(nc, tc, b, wd, xT_d, orix_d, xmask_d, out_d,
                         (pwT, pwsl, diag), ident, inwT, wv, owT, ffcT, peTp, G, E, C_t,
                         inb_cols, outb_col, ffcb_col, db_cols, pb_cols, dwc,
                         xpool, apool, mpool, qkpool, epool, opool,
                         pp, pa, mask_any, bias_any, pad_any)
            for b in range(BS)
        ]
        done = [False] * BS
        last = ["f"] * BS

        def step(i):
            try:
                last[i] = next(gens[i])
            except StopIteration:
                done[i] = True

        step(0)
        emit_consts()
        emit_wall_bulk()
        while not done[0] and last[0] == "f":
            step(0)
        for b in range(BS):
            nxt = b + 1 if b + 1 < BS else None
            while not done[b]:
                step(b)
                if nxt is not None and not done[nxt] and last[nxt] == "f":
                    step(nxt)


def _trace_batch(nc, tc, b, wd, xT_d, orix_d, xmask_d, out_d,
                 pw_pack, ident, inwT, wv, owT, ffcT, peTp, G, E, C_t,
                 inb_cols, outb_col, ffcb_col, db_cols, pb_cols, dwc,
                 xpool, apool, mpool, qkpool, epool, opool,
                 pp, pa, mask_any, bias_any, pad_any):
    pwT, pwsl, diag = pw_pack
    XWT = CT * TB
    AWT = CT * T

    def tap_sh(k):
        s_ = k - K // 2
        return s_, max(0, -s_), T - max(0, s_)

    def xs(w, ct):
        """halo'd block slice [128, T] of a [128, CT*TB] tile"""
        return w[:, TB * ct + 4:TB * ct + 4 + T]

    def asl(w, ct):
        """block slice of a packed [128, CT*T] tile"""
        return w[:, T * ct:T * (ct + 1)]

    def halo_memset(w):
        """zero block-0's halo bands (the only halo the PE diag units read)"""
        a = w[:].bitcast(F32)
        nc.vector.memset(bass.AP(a.tensor, a.offset, [a.ap[0], [516, 2], [1, 4]]),
                         0.0)

    # ---------------- pos_emb + input load ----------------
    if pad_any:
        mrow = mpool.tile([1, T], mybir.dt.int32, tag="mrow_i", name="mrow_i")
        nc.gpsimd.dma_start(mrow[:], orix_d[b:b + 1, :])
        mrow_f = mpool.tile([1, T], F32, tag="mrow_f", name="mrow_f")
        nc.vector.tensor_copy(mrow_f[:], mrow[:])
        nc.vector.tensor_scalar_min(mrow_f[:], mrow_f[:], 1.0)
        m_bc = mpool.tile([128, T], F32, tag="m_bc", name="m_bc", bufs=1)
        nc.gpsimd.partition_broadcast(m_bc[:], mrow_f[:])
    xinw = xpool.tile([128, XWT], F32, tag="xin", name="xin", bufs=1)
    nc.gpsimd.memset(xinw[96:128, 3 * TB + 4:3 * TB + 4 + T], 0.0)
    for ct in range(CT):
        r = _rows(ct)
        nc.sync.dma_start(xs(xinw, ct)[0:r, :], xT_d[b, 128 * ct:128 * ct + r, :])
    xw = xpool.tile([128, XWT], F32R, tag="xw", name="xw")
    halo_memset(xw)
    if pad_any:
        pemw = mpool.tile([128, XWT], F32, tag="pem", name="pem", bufs=1)
        mb = m_bc[:]
        m_rep = bass.AP(mb.tensor, mb.offset, [mb.ap[0], [0, CT], [1, T]])
        pv = pemw[:]
        pem3 = bass.AP(pv.tensor, pv.offset + 4, [pv.ap[0], [TB, CT], [1, T]])
        pe_ = peTp[:]
        pe3 = bass.AP(pe_.tensor, pe_.offset + 4, [pe_.ap[0], [TB, CT], [1, T]])
        nc.gpsimd.tensor_tensor(pem3, pe3, m_rep, op=ALU.mult)
        for ct in range(CT):
            nc.gpsimd.tensor_tensor(xs(xw, ct), xs(xinw, ct), xs(pemw, ct),
                                    op=ALU.add)
    else:
        for ct in range(CT):
            nc.gpsimd.tensor_tensor(xs(xw, ct), xs(xinw, ct),
                                    peTp[:, TB * ct + 4:TB * ct + 4 + T],
                                    op=ALU.add)

    yield "f"

    # ---------------- 4x sepconv (+ residual via PE identity) ----------------
    ACC_DT = FP8 if PW_FP8 else BF16
    for l in range(4):
        def wcol(k, blk):
            o = (l * CT + blk) * K + k
            return dwc[:, o:o + 1]

        accd = apool.tile([128, AWT], BF16, tag="accd", name="accd")
        acc8 = (apool.tile([128, AWT], FP8, tag="acc8", name="acc8")
                if PW_FP8 else accd)
        if POOL_TAPS[l]:
            accp = apool.tile([128, AWT], BF16, tag="accp", name="accp")
        ppw = {}

        def pw_mm(ot, blk_or_pr, start):
            if PW_FP8:
                pr = blk_or_pr
                rhs = acc8[:, 1024 * pr:1024 * (pr + 1)].rearrange(
                    "p (two n) -> p two n", two=2)
                nc.tensor.matmul(ppw[ot][:], pwsl(l, pr, ot), rhs,
                                 start=start, stop=False,
                                 perf_mode=mybir.MatmulPerfMode.DoubleRow)
            else:
                blk = blk_or_pr
                nc.tensor.matmul(ppw[ot][:],
                                 pwT[l][blk][:, 128 * ot:128 * (ot + 1)],
                                 asl(accd, blk), start=start, stop=False)

        # block-major: each contraction block's taps finish, then its (or its
        # pair's) pointwise matmuls start while later blocks' taps still run
        for blk in range(CT):
            ad = asl(accd, blk)
            a8 = asl(acc8, blk)
            xb = xs(xw, blk)
            if (l, blk) in PE_UNITS:
                pdw = pp.tile([128, T], F32, tag="ps", name="ps")
                for j, k in enumerate(range(K)):
                    s_, lo, hi = tap_sh(k)
                    nc.tensor.matmul(pdw[:], diag[(l, blk)][k][:],
                                     xw[:, TB * blk + 4 + s_:TB * blk + 4 + s_ + T],
                                     start=(j == 0), stop=(j == K - 1),
                                     skip_group_check=True)
                nc.scalar.activation(a8, pdw[:], AF.Identity)
            else:
                for i, k in enumerate(DVE_TAPS[l]):
                    s_, lo, hi = tap_sh(k)
                    if i == 0:
                        nc.vector.tensor_scalar_mul(ad, xb, wcol(k, blk))
                    else:
                        nc.vector.scalar_tensor_tensor(
                            ad[:, lo:hi], xb[:, lo + s_:hi + s_], wcol(k, blk),
                            ad[:, lo:hi], op0=ALU.mult, op1=ALU.add)
                if POOL_TAPS[l]:
                    ap_ = asl(accp, blk)
                    for i, k in enumerate(POOL_TAPS[l]):
                        s_, lo, hi = tap_sh(k)
                        wb = wcol(k, blk).broadcast_to([128, hi - lo])
                        if i == 0:
                            nc.gpsimd.tensor_tensor(ap_[:, lo:hi],
                                                    xb[:, lo + s_:hi + s_], wb,
                                                    op=ALU.mult)
                        else:
                            z = apool.tile([128, T], BF16, tag="pz", name="pz",
                                           bufs=2)
                            nc.gpsimd.tensor_tensor(z[:, lo:hi],
                                                    xb[:, lo + s_:hi + s_], wb,
                                                    op=ALU.mult)
                            nc.gpsimd.tensor_tensor(ap_[:, lo:hi], z[:, lo:hi],
                                                    ap_[:, lo:hi], op=ALU.add)
                    _, lo0, hi0 = tap_sh(POOL_TAPS[l][0])
                    nc.vector.tensor_tensor(a8[:, lo0:hi0], ad[:, lo0:hi0],
                                            ap_[:, lo0:hi0], op=ALU.add)
                    if PW_FP8 and lo0 > 0:
                        nc.vector.tensor_copy(a8[:, 0:lo0], ad[:, 0:lo0])
                    if PW_FP8 and hi0 < T:
                        nc.vector.tensor_copy(a8[:, hi0:T], ad[:, hi0:T])
                elif PW_FP8:
                    nc.vector.tensor_copy(a8, ad)
            yield "f"
        xw2 = xpool.tile([128, XWT], F32R, tag="xw", name="xw")
        if l < 2:
            halo_memset(xw2)

        def fin(ot):
            nc.tensor.matmul(ppw[ot][:], ident[:], xs(xw, ot),
                             start=False, stop=True)
            if bias_any:
                nc.scalar.activation(xs(xw2, ot), ppw[ot][:], AF.Identity,
                                     bias=pb_cols[l][:, ot:ot + 1])
            else:
                nc.scalar.activation(xs(xw2, ot), ppw[ot][:], AF.Identity)

        for pair in ((0, 1), (2, 3)):
            for ot in pair:
                ppw[ot] = pp.tile([128, T], F32, tag="ps", name="ps")
                if PW_FP8:
                    for pr in (0, 1):
                        pw_mm(ot, pr, start=(pr == 0))
                else:
                    for blk in range(CT):
                        pw_mm(ot, blk, start=(blk == 0))
            yield "f"
            fin(pair[0])
            fin(pair[1])
        xw = xw2
        yield "f"

    # ---------------- attention ----------------
    # q (p=0..4) and k (p=5..9) pair tiles
    qk = []
    for p in range(10):
        pq = pp.tile([128, T], F32, tag="ps", name="ps")
        for ct in range(CT):
            nc.tensor.matmul(pq[:], inwT[ct][:, 128 * p:128 * (p + 1)],
                             xs(xw, ct), start=(ct == 0), stop=(ct == CT - 1))
        qt = qkpool.tile([128, T], BF16, tag=f"qk{p}", name=f"qk{p}", bufs=2)
        if bias_any:
            nc.scalar.activation(qt[:], pq[:], AF.Identity,
                                 bias=inb_cols[:, p:p + 1])
        else:
            nc.scalar.activation(qt[:], pq[:], AF.Identity)
        qk.append(qt)
        if p % 3 == 2:
            yield "f"
    # v^T, 4 kt tiles of [128, 640] (psum split 384+256)
    vaug = []
    for kt in range(CT):
        pv0 = pp.tile([128, 384], F32, tag="ps", name="ps")
        pv1 = pp.tile([128, HP - 384], F32, tag="ps", name="ps")
        for ct in range(CT):
            xst = xw[:, TB * ct + 4 + 128 * kt:TB * ct + 4 + 128 * (kt + 1)]
            nc.tensor.matmul(pv0[:], xst, wv[ct][:, 0:384],
                             start=(ct == 0), stop=(ct == CT - 1))
            nc.tensor.matmul(pv1[:], xst, wv[ct][:, 384:HP],
                             start=(ct == 0), stop=(ct == CT - 1))
        vt = qkpool.tile([128, HP], BF16, tag=f"vaug{kt}", name=f"vaug{kt}", bufs=2)
        nc.scalar.activation(vt[:, 0:384], pv0[:], AF.Identity)
        nc.scalar.activation(vt[:, 384:HP], pv1[:], AF.Identity)
        if bias_any:
            nc.vector.tensor_tensor(vt[:], vt[:], C_t[:], op=ALU.add)
        else:
            # ones column at slot 50 of each head for the softmax sums
            ones_ap = bass.AP(vt[:].tensor, vt[:].offset + 50,
                              [vt[:].ap[0], [64, H], [1, 1]])
            nc.gpsimd.memset(ones_ap, 1.0)
        vaug.append(vt)
        if kt % 2 == 1:
            yield "f"
    # attention mask multiplier (only traced when mask is nonzero)
    keep = None
    if mask_any:
        keep = []
        for kt in range(CT):
            kc_u8 = mpool.tile([128, 1], mybir.dt.uint8, tag=f"kc8_{kt}", name=f"kc8_{kt}")
            nc.sync.dma_start(
                kc_u8[:],
                xmask_d[b, 128 * kt:128 * (kt + 1)].rearrange(
                    "(t one) -> t one", one=1))
            kc = mpool.tile([128, 1], F32, tag=f"kc{kt}", name=f"kc{kt}")
            nc.vector.tensor_copy(kc[:], kc_u8[:])
            nc.vector.tensor_scalar(kc[:], kc[:], -1.0, 1.0,
                                    op0=ALU.mult, op1=ALU.add)
            keep.append(kc)

    abuf = []
    pats = {}
    expts = {}
    halves = [(p, h) for p in range(5) for h in (2 * p, 2 * p + 1)]

    def emit_scores(p, h):
        sh = 64 * (h % 2)
        expt = []
        for m in range(CT):
            ps_ = pp.tile([128, T], F32, tag="ps", name="ps")
            nc.tensor.matmul(ps_[:], qk[5 + p][sh:sh + 64, 128 * m:128 * (m + 1)],
                             qk[p][sh:sh + 64, :], start=True, stop=True)
            et = epool.tile([128, T], BF16, tag=f"exp{m}", name=f"exp{m}")
            nc.scalar.activation(et[:], ps_[:], AF.Exp)
            if keep is not None:
                nc.vector.tensor_scalar_mul(et[:], et[:], keep[m][:])
            expt.append(et)
        expts[(p, h)] = expt

    def emit_av(p, h):
        if p not in pats:
            pats[p] = pa.tile([128, T], F32, tag="pat", name="pat", bufs=2)
        pat = pats[p]
        sh = 64 * (h % 2)
        expt = expts.pop((p, h))
        for m in range(CT):
            nc.tensor.matmul(pat[sh:sh + 64, :],
                             vaug[m][:, 128 * p + sh:128 * p + sh + 64],
                             expt[m][:], start=(m == 0), stop=(m == CT - 1))
        if h % 2 == 1:
            ab = qkpool.tile([128, T], BF16, tag=f"abuf{p}", name=f"abuf{p}", bufs=1)
            nc.scalar.activation(ab[:], pats.pop(p)[:], AF.Identity)
            abuf.append(ab)

    # two-half-head software lookahead keeps the PE fed across the
    # scores -> exp (ACT) -> attn-weighted-sum dependency hop
    LOOK = 3
    for i in range(len(halves) + LOOK):
        if i < len(halves):
            emit_scores(*halves[i])
        if i >= LOOK:
            emit_av(*halves[i - LOOK])
        yield "b"
    pr = pp.tile([H, T], F32, tag="ps", name="ps")
    for p in range(5):
        nc.tensor.matmul(pr[:], G[p][:], abuf[p][:],
                         start=(p == 0), stop=(p == 4))
    rrec = mpool.tile([H, T], F32R, tag="rrec", name="rrec", bufs=1)
    with nc.allow_low_precision(reason="f32r carries full fp32 bits; recip of softmax sums"):
        nc.vector.reciprocal(rrec[:], pr[:])
    yield "b"
    anorm = []
    for p in range(5):
        pbc = pp.tile([128, T], F32, tag="ps", name="ps")
        nc.tensor.matmul(pbc[:], E[p][:], rrec[:], start=True, stop=True)
        an = qkpool.tile([128, T], BF16, tag=f"anorm{p}", name=f"anorm{p}", bufs=1)
        nc.vector.tensor_tensor(an[:], abuf[p][:], pbc[:], op=ALU.mult)
        anorm.append(an)
    # out-proj + residual via PE identity
    x2w = xpool.tile([128, XWT], F32R, tag="xw", name="xw")
    for ot in range(CT):
        po = pp.tile([128, T], F32, tag="ps", name="ps")
        for p in range(5):
            nc.tensor.matmul(po[:], owT[p][:, 128 * ot:128 * (ot + 1)],
                             anorm[p][:], start=(p == 0), stop=False)
        nc.tensor.matmul(po[:], ident[:], xs(xw, ot), start=False, stop=True)
        if bias_any:
            nc.scalar.activation(xs(x2w, ot), po[:], AF.Identity,
                                 bias=outb_col[:, ot:ot + 1])
        else:
            nc.scalar.activation(xs(x2w, ot), po[:], AF.Identity)
    yield "b"

    # ---------------- ffc + residual + store ----------------
    for ot in range(CT):
        pf = pp.tile([128, T], F32, tag="ps", name="ps")
        for ct in range(CT):
            nc.tensor.matmul(pf[:], ffcT[ct][:, 128 * ot:128 * (ot + 1)],
                             xs(x2w, ct), start=(ct == 0), stop=False)
        nc.tensor.matmul(pf[:], ident[:], xs(x2w, ot), start=False, stop=True)
        ott = opool.tile([128, T], F32, tag="out", name="out", bufs=2)
        if bias_any:
            nc.scalar.activation(ott[:], pf[:], AF.Identity,
                                 bias=ffcb_col[:, ot:ot + 1])
        else:
            nc.scalar.activation(ott[:], pf[:], AF.Identity)
        r = _rows(ot)
        nc.sync.dma_start(out_d[b, 128 * ot:128 * ot + r, :], ott[0:r, :])


_CACHE = {}


def _get_program(consts, mask_any, bias_any, pad_any):
    key = (mask_any, bias_any, pad_any)
    if key not in _CACHE:
        _CACHE[key] = trace_program(consts, mask_any, bias_any, pad_any)
    return _CACHE[key]


def kernel(ori_x, x, x_mask,
           dw1, db1, pw1, pb1, dw2, db2, pw2, pb2,
           dw3, db3, pw3, pb3, dw4, db4, pw4, pb4,
           in_w, in_b, out_w, out_b, ffc_w, ffc_b, _results=None):
    ori_x = np.asarray(ori_x)
    x = np.asarray(x, dtype=np.float32)
    x_mask = np.asarray(x_mask)
    consts = build_host_consts(
        [np.asarray(d, np.float32) for d in (dw1, dw2, dw3, dw4)],
        [np.asarray(p, np.float32) for p in (pw1, pw2, pw3, pw4)],
        [np.asarray(d, np.float32) for d in (db1, db2, db3, db4)],
        [np.asarray(p, np.float32) for p in (pb1, pb2, pb3, pb4)],
        np.asarray(in_w, np.float32), np.asarray(in_b, np.float32),
        np.asarray(out_w, np.float32), np.asarray(out_b, np.float32),
        np.asarray(ffc_w, np.float32), np.asarray(ffc_b, np.float32))
    bias_any = any(np.any(np.asarray(v)) for v in
                   (db1, db2, db3, db4, pb1, pb2, pb3, pb4, in_b, out_b, ffc_b))
    mask_any = bool(np.asarray(x_mask).any())
    pad_any = bool((np.asarray(ori_x) == 0).any())
    nc = _get_program(consts, mask_any, bias_any, pad_any)

    xT = np.ascontiguousarray(x.transpose(0, 2, 1))       # [B, D, T]
    ori32 = ori_x.astype(np.int32)
    mask8 = x_mask.astype(np.uint8)
    in_maps = []
    for c in range(NC_):
        sl = slice(BS * c, BS * (c + 1))
        m = {"xT": xT[sl], "orix": ori32[sl], "xmask": mask8[sl]}
        m.update({k: v for k, v in consts.items() if k != "_offs"})
        in_maps.append(m)
    res = run_bass_kernel_spmd(nc, in_maps, list(range(NC_)))
    if _results is not None:
        _results.append(res)
    outT = np.concatenate([res.results[c]["out"] for c in range(NC_)], axis=0)
    return np.ascontiguousarray(outT.transpose(0, 2, 1)).astype(np.float32)
